# revision 1
# baseline (speedup 1.0000x reference)
"""AdaAttN (nn_AdaAttN_7945689497709) Trainium2 kernel.

B=8 batches -> data-parallel across 8 NeuronCores (one batch per core).
Shapes hardcoded: content/content_key [8, 512, 8192], style/style_key
[8, 512, 2048], Wf/Wg/Wh [512, 512], bf/bg/bh [512].

All heavy matmuls run in float32r (TF32-like: full PE rate, ~12-bit
mantissa, fp32 exponent range), which lets exp(logits) be stored
unnormalized (a constant logit shift keeps sum(exp) inside the ACT Ln
LUT's valid range) and keeps the final rel err ~1.7e-3.
"""
import sys

sys.path.insert(0, "/opt/trn_rl_repo")

import numpy as np
from contextlib import ExitStack

import concourse.bass as bass
import concourse.mybir as mybir
import concourse.tile as tile

dt32 = mybir.dt.float32
dtr = mybir.dt.float32r
AF = mybir.ActivationFunctionType
ALU = mybir.AluOpType
EPS = 1e-5

B, C, T_C, T_S = 8, 512, 8192, 2048


def _split_multi_waits(nc, max_waits=1):
    """This container's walrus rejects >1 sem wait per instruction."""
    n_new = 0
    for f in nc.m.functions:
        for blk in f.blocks:
            insts = blk.instructions
            i = 0
            while i < len(insts):
                inst = insts[i]
                si = inst.sync_info
                if si is not None and si.on_wait and len(si.on_wait) > max_waits:
                    extra = si.on_wait[:-max_waits]
                    keep = si.on_wait[-max_waits:]
                    nops = []
                    for w in extra:
                        n_new += 1
                        nop = mybir.InstNoOp(name=f"I-waitsplit-{n_new}", ins=[], outs=[])
                        nop.engine = inst.engine
                        nop.sync_info = mybir.SyncInfo(on_wait=[w], on_update=[])
                        nops.append(nop)
                    si.on_wait = keep
                    insts[i:i] = nops
                    i += len(nops)
                i += 1
    return n_new


def build_adaattn(C=512, Ts=2048, Tc=8192, TB=512):
    assert C == 512 and TB == 512
    CCH = C // 128          # 4
    SCH = Ts // 128         # 16
    NBLK = Tc // TB

    nc = bass.Bass()
    content = nc.declare_dram_parameter("content", [C, Tc], dt32, isOutput=False)
    style = nc.declare_dram_parameter("style", [C, Ts], dt32, isOutput=False)
    ck = nc.declare_dram_parameter("content_key", [C, Tc], dt32, isOutput=False)
    sk = nc.declare_dram_parameter("style_key", [C, Ts], dt32, isOutput=False)
    Wf = nc.declare_dram_parameter("Wf", [C, C], dt32, isOutput=False)
    bf = nc.declare_dram_parameter("bf", [C], dt32, isOutput=False)
    Wg = nc.declare_dram_parameter("Wg", [C, C], dt32, isOutput=False)
    bg = nc.declare_dram_parameter("bg", [C], dt32, isOutput=False)
    WhT = nc.declare_dram_parameter("WhT", [C, C], dt32, isOutput=False)
    bh = nc.declare_dram_parameter("bh", [C], dt32, isOutput=False)
    out = nc.declare_dram_parameter("out", [C, Tc], dt32, isOutput=True)

    def chunked(h):
        return h[:].rearrange("(cc p) t -> p cc t", p=128)

    content_v, ck_v, out_v = chunked(content), chunked(ck), chunked(out)

    def b2(t, k):
        return t[:, 2 * k:2 * k + 2]

    with ExitStack() as ctx:
        tc = ctx.enter_context(tile.TileContext(nc))
        persist = ctx.enter_context(tc.tile_pool(name="persist", bufs=1))

        # -------- persistent tiles (resident for whole kernel) --------
        Ghat_r = [persist.tile([128, Ts], dtr, tag=f"Ghat{i}", name=f"Ghat{i}")
                  for i in range(CCH)]
        HT_r = [persist.tile([128, C], dtr, tag=f"HT{s}", name=f"HT{s}")
                for s in range(SCH)]
        Hsq_r = [persist.tile([128, C], dtr, tag=f"Hsq{s}", name=f"Hsq{s}")
                 for s in range(SCH)]
        vw_t = persist.tile([128, SCH], dt32, tag="vw_t")
        cm_t = persist.tile([128, CCH], dt32, tag="cm_t")
        nhalf_t = persist.tile([128, CCH], dt32, tag="nhalf_t")
        ones128_f = persist.tile([128, 1], dt32, tag="ones128f")
        nc.vector.memset(ones128_f[:], 1.0)
        ones128_r = persist.tile([128, 1], dtr, tag="ones128")
        nc.vector.tensor_copy(ones128_r[:], ones128_f[:])
        ones1_f = persist.tile([1, 128], dt32, tag="ones1f")
        nc.vector.memset(ones1_f[:], 1.0)
        ones1_r = persist.tile([1, 128], dtr, tag="ones1")
        nc.vector.tensor_copy(ones1_r[:], ones1_f[:])

        # ================= PROLOGUE =================
        with tc.tile_pool(name="proS", bufs=1) as proS:
            bf32_t = proS.tile([128, CCH], dt32, tag="bf32")
            bfr_t = proS.tile([128, 2 * CCH], dtr, tag="bfr")
            bg32_t = proS.tile([128, CCH], dt32, tag="bg32")
            bgr_t = proS.tile([128, 2 * CCH], dtr, tag="bgr")
            u_t = proS.tile([128, CCH], dt32, tag="u_t")
            y_t = proS.tile([128, 2 * CCH], dtr, tag="y_t")
            wb = proS.tile([128, 1], dt32, tag="wb")
            PT_r = [proS.tile([128, C], dtr, tag=f"PT{j}", name=f"PT{j}")
                    for j in range(CCH)]

            for (dram, f32t, rt) in ((bf, bf32_t, bfr_t), (bg, bg32_t, bgr_t)):
                for k in range(CCH):
                    nc.scalar.dma_start(
                        f32t[:, k:k + 1],
                        dram[k * 128:(k + 1) * 128].rearrange("(p o) -> p o", o=1))
                for two in range(2):
                    nc.vector.tensor_copy(
                        rt[:].rearrange("p (k two) -> p k two", two=2)[:, :, two], f32t[:])

            # ---- scope B1: Wf/Wg -> PT, u, y, w ----
            with (
                tc.tile_pool(name="proB", bufs=1) as proB,
                tc.tile_pool(name="proBs", bufs=2) as proBs,
                tc.tile_pool(name="proB_ps", bufs=3, space="PSUM") as psB_,
                tc.tile_pool(name="proB_ps1", bufs=2, space="PSUM") as psB1_,
            ):
                Wf_r, Wg_r = [], []
                for k in range(CCH):
                    for dram, rt in ((Wf, Wf_r), (Wg, Wg_r)):
                        stg = proBs.tile([128, C], dt32, tag="wstage",
                                         name="wstage", bufs=4)
                        nc.sync.dma_start(stg[:], chunked(dram)[:, k, :])
                        t = proB.tile([128, C], dtr, tag=f"r_{dram.name}{k}",
                                      name=f"r_{dram.name}{k}")
                        nc.vector.tensor_copy(t[:], stg[:])
                        rt.append(t)

                for j in range(CCH):
                    ps = psB_.tile([128, C], dt32, tag="psPT")
                    for k in range(CCH):
                        nc.tensor.matmul(ps[:], Wg_r[k][:, j * 128:(j + 1) * 128],
                                         Wf_r[k][:], start=(k == 0), stop=(k == CCH - 1))
                    nc.vector.tensor_copy(PT_r[j][:], ps[:])

                for i in range(CCH):
                    ps = psB1_.tile([128, 2], dt32, tag="pss", name="psu")
                    for k in range(CCH):
                        nc.tensor.matmul(ps[:], Wf_r[k][:, i * 128:(i + 1) * 128],
                                         b2(bgr_t, k), start=(k == 0),
                                         stop=(k == CCH - 1))
                    nc.vector.tensor_copy(u_t[:, i:i + 1], ps[:, 0:1])

                for j in range(CCH):
                    ps = psB1_.tile([128, 2], dt32, tag="pss", name="psy")
                    for k in range(CCH):
                        nc.tensor.matmul(ps[:], Wg_r[k][:, j * 128:(j + 1) * 128],
                                         b2(bfr_t, k), start=(k == 0),
                                         stop=(k == CCH - 1))
                    for two in range(2):
                        nc.vector.tensor_copy(y_t[:, 2 * j + two:2 * j + two + 1],
                                              ps[:, 0:1])

                psw = psB1_.tile([1, 2], dt32, tag="pss", name="psw")
                for k in range(CCH):
                    nc.tensor.matmul(psw[:], bfr_t[:, 2 * k:2 * k + 1], b2(bgr_t, k),
                                     start=(k == 0), stop=(k == CCH - 1))
                wsb_r = proBs.tile([1, 2], dtr, tag="wsbr")
                for two in range(2):
                    nc.vector.tensor_copy(wsb_r[:, two:two + 1], psw[:, 0:1])
                pswb = psB1_.tile([128, 2], dt32, tag="pss", name="pswb")
                nc.tensor.matmul(pswb[:], ones1_r[:], wsb_r[:], start=True, stop=True)
                nc.vector.tensor_copy(wb[:], pswb[:, 0:1])

            # ---- scope A: WhT + style -> HT, Hsq ----
            with (
                tc.tile_pool(name="proA", bufs=1) as proA,
                tc.tile_pool(name="proAs", bufs=2) as proAs,
                tc.tile_pool(name="proA_ps", bufs=3, space="PSUM") as psA_,
            ):
                bh_row = proAs.tile([1, C], dt32, tag="bhrow")
                nc.sync.dma_start(bh_row[:], bh[:].rearrange("(o f) -> o f", o=1))
                bh_row_r = proAs.tile([1, C], dtr, tag="bhrowr")
                nc.vector.tensor_copy(bh_row_r[:], bh_row[:])

                WhT_r = []
                for k in range(CCH):
                    stg = proAs.tile([128, C], dt32, tag="whstage", name="whstage")
                    nc.sync.dma_start(stg[:], chunked(WhT)[:, k, :])
                    t = proA.tile([128, C], dtr, tag=f"WhT{k}", name=f"WhT{k}")
                    nc.vector.tensor_copy(t[:], stg[:])
                    WhT_r.append(t)
                style_r = []
                for k in range(CCH):
                    stg = proAs.tile([128, Ts], dt32, tag="sstage", name="sstage", bufs=3)
                    nc.sync.dma_start(stg[:], chunked(style)[:, k, :])
                    t = proA.tile([128, Ts], dtr, tag=f"style_r{k}", name=f"style_r{k}")
                    nc.vector.tensor_copy(t[:], stg[:])
                    style_r.append(t)

                for s in range(SCH):
                    ps = psA_.tile([128, C], dt32, tag="psH")
                    for k in range(CCH):
                        nc.tensor.matmul(ps[:], style_r[k][:, s * 128:(s + 1) * 128],
                                         WhT_r[k][:], start=(k == 0), stop=False)
                    nc.tensor.matmul(ps[:], ones1_r[:], bh_row_r[:], start=False,
                                     stop=True)
                    nc.vector.tensor_copy(HT_r[s][:], ps[:])
                    nc.scalar.activation(Hsq_r[s][:], ps[:], AF.Square)

            # ---- scope B2: sk -> Ghat'', vw ----
            with (
                tc.tile_pool(name="proC", bufs=1) as proC,
                tc.tile_pool(name="proCs", bufs=2) as proCs,
                tc.tile_pool(name="proC_ps", bufs=3, space="PSUM") as psC_,
                tc.tile_pool(name="proC_ps1", bufs=2, space="PSUM") as psC1_,
            ):
                sk_r = []
                for k in range(CCH):
                    stg = proCs.tile([128, Ts], dt32, tag="skstage", name="skstage", bufs=3)
                    nc.sync.dma_start(stg[:], chunked(sk)[:, k, :])
                    t = proC.tile([128, Ts], dtr, tag=f"sk_r{k}", name=f"sk_r{k}")
                    nc.vector.tensor_copy(t[:], stg[:])
                    sk_r.append(t)

                for i in range(CCH):
                    for s4 in range(Ts // 512):
                        ps = psC_.tile([128, 512], dt32, tag="psG")
                        for k in range(CCH):
                            nc.tensor.matmul(ps[:], PT_r[k][:, i * 128:(i + 1) * 128],
                                             sk_r[k][:, s4 * 512:(s4 + 1) * 512],
                                             start=(k == 0), stop=(k == CCH - 1))
                        nc.vector.tensor_scalar(
                            Ghat_r[i][:, s4 * 512:(s4 + 1) * 512], ps[:],
                            u_t[:, i:i + 1], None, op0=ALU.add)

                for s in range(SCH):
                    ps = psC1_.tile([128, 2], dt32, tag="psv")
                    for k in range(CCH):
                        nc.tensor.matmul(ps[:], sk_r[k][:, s * 128:(s + 1) * 128],
                                         b2(y_t, k), start=(k == 0),
                                         stop=(k == CCH - 1))
                    # -K0 keeps r = sum(exp) inside the ACT Ln LUT's valid
                    # input range (~[1e-18, 1e18]); cancels in the softmax.
                    nc.vector.scalar_tensor_tensor(vw_t[:, s:s + 1], ps[:, 0:1],
                                                   -28.0, wb[:], op0=ALU.add,
                                                   op1=ALU.add)

        # ================= MAIN BLOCKS =================
        with (
            tc.tile_pool(name="ckf_p", bufs=3) as ckf_p,
            tc.tile_pool(name="ckr_p", bufs=6) as ckr_p,
            tc.tile_pool(name="ctf_p", bufs=5) as ctf_p,
            tc.tile_pool(name="outb_p", bufs=4) as outb_p,
            tc.tile_pool(name="epool", bufs=SCH + 1) as epool,
            tc.tile_pool(name="dmean", bufs=3) as dmean,
            tc.tile_pool(name="dtmp", bufs=3) as dtmp,
            tc.tile_pool(name="drv", bufs=2) as drv,
            tc.tile_pool(name="ps_a", bufs=3, space="PSUM") as ps_a,
            tc.tile_pool(name="ps_r", bufs=1, space="PSUM") as ps_r,
            tc.tile_pool(name="ps_n", bufs=3, space="PSUM") as ps_n,
        ):
            def emit_stats():
                # ---- scope D: content stats ----
                with tc.tile_pool(name="proD", bufs=2) as proD:
                    QS = 1024 if Tc % 1024 == 0 else Tc
                    NQ = Tc // QS
                    for c in range(CCH):
                        px = proD.tile([128, NQ], dt32, tag="px")
                        px2 = proD.tile([128, NQ], dt32, tag="px2")
                        for h in range(NQ):
                            stg = proD.tile([128, QS], dt32, tag="ctstage", name="ctstage")
                            [nc.gpsimd, nc.sync, nc.scalar][(c * NQ + h) % 3].dma_start(
                                stg[:], content_v[:, c, h * QS:(h + 1) * QS])
                            nc.vector.tensor_reduce(px[:, h:h + 1], stg[:],
                                                    axis=mybir.AxisListType.X, op=ALU.add)
                            trash = proD.tile([128, QS], dt32, tag="trash", bufs=1)
                            nc.scalar.activation(trash[:], stg[:], AF.Square,
                                                 accum_out=px2[:, h:h + 1])
                        sx = proD.tile([128, 1], dt32, tag="sx")
                        nc.vector.tensor_reduce(sx[:], px[:], axis=mybir.AxisListType.X,
                                                op=ALU.add)
                        sx2 = proD.tile([128, 1], dt32, tag="sx2")
                        nc.vector.tensor_reduce(sx2[:], px2[:], axis=mybir.AxisListType.X,
                                                op=ALU.add)
                        nc.vector.tensor_scalar(cm_t[:, c:c + 1], sx[:], 1.0 / Tc, None,
                                                op0=ALU.mult)
                        t1 = proD.tile([128, 1], dt32, tag="t1")
                        nc.vector.tensor_tensor(t1[:], sx[:], sx[:], op=ALU.mult)
                        t2 = proD.tile([128, 1], dt32, tag="t2")
                        nc.vector.scalar_tensor_tensor(t2[:], t1[:], -1.0 / Tc, sx2[:],
                                                       op0=ALU.mult, op1=ALU.add)
                        varc = proD.tile([128, 1], dt32, tag="varc")
                        nc.vector.tensor_scalar(varc[:], t2[:], 1.0 / (Tc - 1), EPS,
                                                op0=ALU.mult, op1=ALU.add)
                        lnv = proD.tile([128, 1], dt32, tag="lnv")
                        nc.scalar.activation(lnv[:], varc[:], AF.Ln)
                        nc.vector.tensor_scalar(nhalf_t[:, c:c + 1], lnv[:], -0.5, None,
                                                op0=ALU.mult)


            for b in range(NBLK):
                t0 = b * TB
                ck_b = []
                for k in range(CCH):
                    stg = ckf_p.tile([128, TB], dt32, tag="ckf", name="ckf")
                    nc.sync.dma_start(stg[:], ck_v[:, k, t0:t0 + TB])
                    t = ckr_p.tile([128, TB], dtr, tag="ckr", name="ckr")
                    nc.vector.tensor_copy(t[:], stg[:])
                    ck_b.append(t)
                ct_f = []
                for k in range(CCH):
                    t = ctf_p.tile([128, TB], dt32, tag="ctf", name="ctf")
                    nc.sync.dma_start(t[:], content_v[:, k, t0:t0 + TB])
                    ct_f.append(t)

                if b == 0:
                    emit_stats()

                # ---- A^T tiles + exp
                E = []
                for s in range(SCH):
                    psA = ps_a.tile([128, TB], dt32, tag="psA", name="psA")
                    for k in range(CCH):
                        nc.tensor.matmul(psA[:], Ghat_r[k][:, s * 128:(s + 1) * 128],
                                         ck_b[k][:], start=(k == 0),
                                         stop=(k == CCH - 1))
                    e = epool.tile([128, TB], dtr, tag="E", name="E")
                    nc.scalar.activation(e[:], psA[:], AF.Exp, bias=vw_t[:, s:s + 1])
                    E.append(e)

                # ---- r = 1^T E ; rinv = exp(-ln r), broadcast via K=1 matmul
                psR = ps_r.tile([1, TB], dt32, tag="psR")
                for s in range(SCH):
                    nc.tensor.matmul(psR[:], ones128_r[:], E[s][:],
                                     start=(s == 0), stop=(s == SCH - 1))
                lnr = drv.tile([1, TB], dt32, tag="lnr", bufs=1)
                nc.scalar.activation(lnr[:], psR[:], AF.Ln)
                rinv_r = drv.tile([1, TB], dtr, tag="rinvr", bufs=1)
                nc.scalar.activation(rinv_r[:], lnr[:], AF.Exp, scale=-1.0)
                psRB = ps_r.tile([128, TB], dt32, tag="psRB", bufs=1)
                nc.tensor.matmul(psRB[:], ones1_r[:], rinv_r[:], start=True, stop=True)
                rinvb = drv.tile([128, TB], dt32, tag="rinvb")
                nc.vector.tensor_copy(rinvb[:], psRB[:])

                # ---- num1/num2 per channel chunk + drain
                for c in range(CCH):
                    ps1 = ps_n.tile([128, TB], dt32, tag="psN", name="psN1")
                    for s in range(SCH):
                        nc.tensor.matmul(ps1[:], HT_r[s][:, c * 128:(c + 1) * 128],
                                         E[s][:], start=(s == 0), stop=(s == SCH - 1))
                    ps2 = ps_n.tile([128, TB], dt32, tag="psN", name="psN2")
                    for s in range(SCH):
                        nc.tensor.matmul(ps2[:], Hsq_r[s][:, c * 128:(c + 1) * 128],
                                         E[s][:], start=(s == 0), stop=(s == SCH - 1))

                    mean = dmean.tile([128, TB], dt32, tag="mean", name="mean")
                    nc.vector.tensor_tensor(mean[:], ps1[:], rinvb[:], op=ALU.mult)
                    sec = dtmp.tile([128, TB], dt32, tag="tmpa", name="sec")
                    nc.vector.tensor_tensor(sec[:], ps2[:], rinvb[:], op=ALU.mult)
                    msq = dtmp.tile([128, TB], dt32, tag="tmpb", name="msq")
                    nc.scalar.activation(msq[:], mean[:], AF.Square)
                    var = dtmp.tile([128, TB], dt32, tag="tmpa", name="var")
                    nc.vector.scalar_tensor_tensor(var[:], msq[:], -1.0, sec[:],
                                                   op0=ALU.mult, op1=ALU.add)
                    varcl = dtmp.tile([128, TB], dt32, tag="tmpb", name="varcl")
                    nc.vector.tensor_scalar(varcl[:], var[:], 1e-18, None, op0=ALU.max)
                    lnvb = dtmp.tile([128, TB], dt32, tag="tmpa", name="lnvb")
                    nc.scalar.activation(lnvb[:], varcl[:], AF.Ln)
                    stdp = dtmp.tile([128, TB], dt32, tag="tmpb", name="stdp")
                    nc.scalar.activation(stdp[:], lnvb[:], AF.Exp,
                                         bias=nhalf_t[:, c:c + 1], scale=0.5)
                    tt = dtmp.tile([128, TB], dt32, tag="tmpa", name="tt")
                    nc.vector.scalar_tensor_tensor(tt[:], ct_f[c][:], cm_t[:, c:c + 1],
                                                   stdp[:], op0=ALU.subtract,
                                                   op1=ALU.mult)
                    ob = outb_p.tile([128, TB], dt32, tag="outb", name="outb")
                    nc.vector.tensor_tensor(ob[:], tt[:], mean[:], op=ALU.add)
                    nc.scalar.dma_start(out_v[:, c, t0:t0 + TB], ob[:])

    _split_multi_waits(nc)
    return nc


_NC_CACHE = {}


def _get_nc():
    if "nc" not in _NC_CACHE:
        _NC_CACHE["nc"] = build_adaattn(C=C, Ts=T_S, Tc=T_C, TB=512)
    return _NC_CACHE["nc"]


def kernel(content, style, content_key, style_key, Wf, bf, Wg, bg, Wh, bh):
    from concourse.bass_utils import run_bass_kernel_spmd

    nc = _get_nc()
    WhT = np.ascontiguousarray(np.asarray(Wh, dtype=np.float32).T)
    shared = {
        "Wf": np.ascontiguousarray(Wf, dtype=np.float32),
        "bf": np.ascontiguousarray(bf, dtype=np.float32),
        "Wg": np.ascontiguousarray(Wg, dtype=np.float32),
        "bg": np.ascontiguousarray(bg, dtype=np.float32),
        "WhT": WhT,
        "bh": np.ascontiguousarray(bh, dtype=np.float32),
    }
    in_maps = []
    for i in range(B):
        m = dict(shared)
        m["content"] = np.ascontiguousarray(content[i], dtype=np.float32)
        m["style"] = np.ascontiguousarray(style[i], dtype=np.float32)
        m["content_key"] = np.ascontiguousarray(content_key[i], dtype=np.float32)
        m["style_key"] = np.ascontiguousarray(style_key[i], dtype=np.float32)
        in_maps.append(m)
    res = run_bass_kernel_spmd(nc, in_maps, core_ids=list(range(B)))
    return np.stack([res.results[i]["out"] for i in range(B)]).astype(np.float32)

